# revision 12
# baseline (speedup 1.0000x reference)
"""MoE (top-2, capacity-dropped, SwiGLU experts) on 8 Trainium2 cores.

Expert-parallel: host computes the (tiny) router + dispatch exactly as the
reference, core e runs the three big GEMMs for expert e in fp16 (fp32
accumulate), host combines with the gate weights.

Self-contained: hardcodes shapes from the problem spec.
"""

import os
import sys

for _p in ("/opt/trn_rl_repo", "/root/.axon_site/_ro/trn_rl_repo"):
    if os.path.isdir(_p) and _p not in sys.path:
        sys.path.append(_p)

import numpy as np

B, T, C, E, H = 8, 2048, 1024, 8, 2752
CAPF = 1.25
N = B * T
CAP = int(CAPF * N * 2 / E)
P = 128
NCT = C // P  # contraction tiles over C
H_TILES = [(i * P, min(P, H - i * P)) for i in range((H + P - 1) // P)]
NHF = H // P  # full 128-row h tiles; the 64-row remainder is handled merged
H_FULL = NHF * P
HREM = H - H_FULL
SCH = 512  # token chunk (matmul free dim / one PSUM bank)


def _route(xf, gate_w):
    """Replicates the reference router in fp32 numpy."""
    logits = xf @ gate_w  # [N, E] fp32
    m = logits.max(axis=-1, keepdims=True)
    eg = np.exp(logits - m)
    gates = eg / eg.sum(axis=-1, keepdims=True)
    i0 = gates.argmax(axis=-1)
    g2 = gates.copy()
    g2[np.arange(N), i0] = -np.inf
    i1 = g2.argmax(axis=-1)
    v0 = gates[np.arange(N), i0]
    v1 = gates[np.arange(N), i1]
    return logits, gates, (i0, i1), (v0, v1)


def _positions(idx):
    """Per-token occurrence index within its chosen expert (token order)."""
    onehot = idx[:, None] == np.arange(E)[None, :]
    return onehot.cumsum(axis=0)[np.arange(N), idx] - 1


def _build(s_bufs, s_chunks):
    import concourse.bacc as bacc
    import concourse.mybir as mybir
    from concourse import tile

    f16 = mybir.dt.float16
    f32 = mybir.dt.float32
    nc = bacc.Bacc(None, target_bir_lowering=False, debug=True)

    xt = nc.dram_tensor("xt", [C, s_bufs], f16, kind="ExternalInput")
    wg = nc.dram_tensor("wg", [C, H], f16, kind="ExternalInput")
    wu = nc.dram_tensor("wu", [C, H], f16, kind="ExternalInput")
    wd = nc.dram_tensor("wd", [H, C], f16, kind="ExternalInput")
    yt = nc.dram_tensor("yt", [C, s_bufs], f32, kind="ExternalOutput")

    nht = len(H_TILES)
    with tile.TileContext(nc) as tc:
        with (
            tc.tile_pool(name="wpool", bufs=1) as wpool,
            tc.tile_pool(name="xpool", bufs=2) as xpool,
            tc.tile_pool(name="apool", bufs=1) as apool,
            tc.tile_pool(name="spool", bufs=2) as spool,
            tc.tile_pool(name="ypool", bufs=4) as ypool,
            tc.tile_pool(name="gup", bufs=2, space="PSUM") as gup,
            tc.tile_pool(name="yp", bufs=1, space="PSUM") as yp,
        ):
            def load_xt_chunk(s0, sw):
                # issued from the scalar queue so startup DMAs don't
                # serialize behind the weight loads on sync (~600 ns/issue)
                tiles = []
                for ct in range(NCT):
                    t = xpool.tile([P, sw], f16, tag=f"x{ct}", name=f"x{ct}")
                    nc.scalar.dma_start(
                        out=t[:], in_=xt[ct * P : (ct + 1) * P, s0 : s0 + sw]
                    )
                    tiles.append(t)
                return tiles

            # PE warm-up: dummy matmuls on a zeroed scratch tile, issued
            # while the first weight/activation DMAs are still in flight so
            # the clock governor un-throttles before real work arrives
            scr = spool.tile([P, 640], f16, tag="warm")
            nc.gpsimd.memset(scr[:], 0.0)
            warm_ps = yp.tile([P, SCH], f32, tag="py0", name="warm")
            for _ in range(40):
                nc.tensor.matmul(
                    warm_ps[:],
                    scr[:, 0:P],
                    scr[:, P : P + SCH],
                    start=True,
                    stop=True,
                    skip_group_check=True,
                )

            # chunk-0 activations first: the first matmul needs them
            xt0 = load_xt_chunk(*s_chunks[0])
            # Wg/Wu in h-ascending slabs (first slab = just h-tile 0) so
            # compute starts after ~1.5 MB of DMA instead of the full 11 MB
            wg_sb = [wpool.tile([P, H_FULL], f16, tag=f"wg{ct}", name=f"wg{ct}") for ct in range(NCT)]
            wu_sb = [wpool.tile([P, H_FULL], f16, tag=f"wu{ct}", name=f"wu{ct}") for ct in range(NCT)]
            slabs, h0 = [], 0
            for hw in (128, 256, 256, 256, 256, 512, 512, 512):
                slabs.append((h0, hw))
                h0 += hw
            assert h0 == H_FULL
            for h0, hw in slabs:
                for ct in range(NCT):
                    nc.sync.dma_start(
                        out=wg_sb[ct][:, h0 : h0 + hw],
                        in_=wg[ct * P : (ct + 1) * P, h0 : h0 + hw],
                    )
                    nc.sync.dma_start(
                        out=wu_sb[ct][:, h0 : h0 + hw],
                        in_=wu[ct * P : (ct + 1) * P, h0 : h0 + hw],
                    )
            # stacked h-remainder: cols 0:64 = Wg[:, H_FULL:], 64:128 = Wu[:, H_FULL:]
            wgu_rem = []
            for ct in range(NCT):
                t = wpool.tile([P, 2 * HREM], f16, tag=f"wr{ct}", name=f"wr{ct}")
                nc.sync.dma_start(
                    out=t[:, 0:HREM], in_=wg[ct * P : (ct + 1) * P, H_FULL:H]
                )
                nc.sync.dma_start(
                    out=t[:, HREM : 2 * HREM],
                    in_=wu[ct * P : (ct + 1) * P, H_FULL:H],
                )
                wgu_rem.append(t)
            wd_sb = []
            for hi, (h0, hsz) in enumerate(H_TILES):
                t = wpool.tile([hsz, C], f16, tag=f"wd{hi}")
                nc.sync.dma_start(out=t[:], in_=wd[h0 : h0 + hsz, :])
                wd_sb.append(t)

            for si, (s0, sw) in enumerate(s_chunks):
                xt_sb = xt0 if si == 0 else load_xt_chunk(s0, sw)
                # --- G/U + SwiGLU epilogue, one h-tile at a time ---
                a_sb = []
                for hi in range(NHF):
                    h0 = hi * P
                    pg = gup.tile([P, SCH], f32, tag="pg")
                    pu = gup.tile([P, SCH], f32, tag="pu")
                    for ct in range(NCT):
                        nc.tensor.matmul(
                            pg[:, :sw],
                            wg_sb[ct][:, h0 : h0 + P],
                            xt_sb[ct][:, :sw],
                            start=(ct == 0),
                            stop=(ct == NCT - 1),
                        )
                    for ct in range(NCT):
                        nc.tensor.matmul(
                            pu[:, :sw],
                            wu_sb[ct][:, h0 : h0 + P],
                            xt_sb[ct][:, :sw],
                            start=(ct == 0),
                            stop=(ct == NCT - 1),
                        )
                    sg = spool.tile([P, sw], f16, tag="sg")
                    nc.scalar.activation(
                        sg[:, :sw],
                        pg[:, :sw],
                        mybir.ActivationFunctionType.Silu,
                    )
                    a = apool.tile([P, sw], f16, tag=f"a{hi}")
                    nc.vector.tensor_mul(a[:, :sw], sg[:, :sw], pu[:, :sw])
                    a_sb.append(a)
                # h remainder: one stacked MM gives G on rows 0:64, U on 64:128
                pgu = gup.tile([P, SCH], f32, tag="pg")
                for ct in range(NCT):
                    nc.tensor.matmul(
                        pgu[:, :sw],
                        wgu_rem[ct][:, :],
                        xt_sb[ct][:, :sw],
                        start=(ct == 0),
                        stop=(ct == NCT - 1),
                    )
                sg = spool.tile([HREM, sw], f16, tag="sgr")
                nc.scalar.activation(
                    sg[:, :sw], pgu[0:HREM, :sw], mybir.ActivationFunctionType.Silu
                )
                a = apool.tile([HREM, sw], f16, tag=f"a{NHF}")
                nc.vector.tensor_mul(
                    a[:, :sw], sg[:, :sw], pgu[HREM : 2 * HREM, :sw]
                )
                a_sb.append(a)
                # --- down-proj: Y^T[c, s] = sum_h Wd[h, c] * A[h, s] ---
                final = si == len(s_chunks) - 1
                for ch in range(2):
                    pys = [
                        yp.tile([P, SCH], f32, tag=f"py{i}", name=f"py{i}")
                        for i in range(4)
                    ]
                    if final and ch == 1:
                        # i-major: each psum group finishes early so its
                        # eviction overlaps the remaining groups' matmuls
                        order = [(hi, i) for i in range(4) for hi in range(nht)]
                    else:
                        order = [(hi, i) for hi in range(nht) for i in range(4)]
                    for hi, i in order:
                        h0, hsz = H_TILES[hi]
                        c0 = ch * 512 + i * P
                        nc.tensor.matmul(
                            pys[i][:, :sw],
                            wd_sb[hi][:hsz, c0 : c0 + P],
                            a_sb[hi][:hsz, :sw],
                            start=(hi == 0),
                            stop=(hi == nht - 1),
                        )
                    for i in range(4):
                        c0 = ch * 512 + i * P
                        ysb = ypool.tile([P, sw], f32, tag="y")
                        nc.vector.tensor_copy(ysb[:, :sw], pys[i][:, :sw])
                        nc.sync.dma_start(
                            out=yt[c0 : c0 + P, s0 : s0 + sw], in_=ysb[:, :sw]
                        )
    nc.compile()
    return nc


def run(inputs, trace=False, tmpdir=None):
    from concourse.bass_utils import run_bass_kernel_spmd

    x = np.ascontiguousarray(np.asarray(inputs["x"], dtype=np.float32))
    gate_w = np.asarray(inputs["gate_w"], dtype=np.float32)
    Wg = np.asarray(inputs["Wg"], dtype=np.float32)
    Wu = np.asarray(inputs["Wu"], dtype=np.float32)
    Wd = np.asarray(inputs["Wd"], dtype=np.float32)

    xf = x.reshape(N, C)
    logits, gates, (i0, i1), (v0, v1) = _route(xf, gate_w)

    # losses (exact host math, fp64 accumulate)
    me = gates.astype(np.float64).mean(axis=0)
    ce = np.bincount(i0, minlength=E).astype(np.float64) / N
    aux_loss = np.float32(E * np.sum(me * ce))
    z_loss = np.float32(np.mean(logits.astype(np.float64) ** 2))

    # dispatch (first-come per k, capacity CAP per (expert, k))
    groups = []  # (tokens, vals) per (k, e)
    counts = np.zeros(E, dtype=np.int64)
    tok_by = {}
    for k, (idx, val) in enumerate(((i0, v0), (i1, v1))):
        pos = _positions(idx)
        keep = pos < CAP
        for e in range(E):
            sel = (idx == e) & keep
            toks = np.nonzero(sel)[0]
            tok_by[(k, e)] = toks
            counts[e] += len(toks)

    s_bufs = max(SCH, int(counts.max()))
    s_chunks = []
    s0 = 0
    while s0 < s_bufs:
        sw = min(SCH, s_bufs - s0)
        s_chunks.append((s0, sw))
        s0 += sw

    xf16 = xf.astype(np.float16)
    in_maps = []
    offsets = {}
    for e in range(E):
        t0 = tok_by[(0, e)]
        t1 = tok_by[(1, e)]
        offsets[e] = (len(t0), len(t1))
        xe = np.zeros((s_bufs, C), np.float16)
        xe[: len(t0)] = xf16[t0]
        xe[len(t0) : len(t0) + len(t1)] = xf16[t1]
        in_maps.append(
            {
                "xt": np.ascontiguousarray(xe.T),
                "wg": Wg[e].astype(np.float16),
                "wu": Wu[e].astype(np.float16),
                "wd": Wd[e].astype(np.float16),
            }
        )

    nc = _build(s_bufs, s_chunks)
    res = run_bass_kernel_spmd(
        nc, in_maps, list(range(E)), trace=trace, tmpdir=tmpdir
    )

    out = np.zeros((N, C), np.float32)
    for e in range(E):
        y = res.results[e]["yt"]  # [C, s_bufs] fp32
        n0, n1 = offsets[e]
        t0 = tok_by[(0, e)]
        t1 = tok_by[(1, e)]
        if n0:
            out[t0] += v0[t0, None] * y[:, :n0].T
        if n1:
            out[t1] += v1[t1, None] * y[:, n0 : n0 + n1].T

    return (out.reshape(B, T, C), aux_loss, z_loss), res


def kernel(**inputs):
    outs, _ = run(inputs)
    return outs


# revision 13
# speedup vs baseline: 1.0200x; 1.0200x over previous
"""MoE (top-2, capacity-dropped, SwiGLU experts) on 8 Trainium2 cores.

Expert-parallel: host computes the (tiny) router + dispatch exactly as the
reference, core e runs the three big GEMMs for expert e in fp16 (fp32
accumulate), host combines with the gate weights.

Self-contained: hardcodes shapes from the problem spec.
"""

import os
import sys

for _p in ("/opt/trn_rl_repo", "/root/.axon_site/_ro/trn_rl_repo"):
    if os.path.isdir(_p) and _p not in sys.path:
        sys.path.append(_p)

import numpy as np

B, T, C, E, H = 8, 2048, 1024, 8, 2752
CAPF = 1.25
N = B * T
CAP = int(CAPF * N * 2 / E)
P = 128
NCT = C // P  # contraction tiles over C
H_TILES = [(i * P, min(P, H - i * P)) for i in range((H + P - 1) // P)]
NHF = H // P  # full 128-row h tiles; the 64-row remainder is handled merged
H_FULL = NHF * P
HREM = H - H_FULL
SCH = 512  # token chunk (matmul free dim / one PSUM bank)


def _route(xf, gate_w):
    """Replicates the reference router in fp32 numpy."""
    logits = xf @ gate_w  # [N, E] fp32
    m = logits.max(axis=-1, keepdims=True)
    eg = np.exp(logits - m)
    gates = eg / eg.sum(axis=-1, keepdims=True)
    i0 = gates.argmax(axis=-1)
    g2 = gates.copy()
    g2[np.arange(N), i0] = -np.inf
    i1 = g2.argmax(axis=-1)
    v0 = gates[np.arange(N), i0]
    v1 = gates[np.arange(N), i1]
    return logits, gates, (i0, i1), (v0, v1)


def _positions(idx):
    """Per-token occurrence index within its chosen expert (token order)."""
    onehot = idx[:, None] == np.arange(E)[None, :]
    return onehot.cumsum(axis=0)[np.arange(N), idx] - 1


def _build(s_bufs, s_chunks):
    import concourse.bacc as bacc
    import concourse.mybir as mybir
    from concourse import tile

    f16 = mybir.dt.float16
    f32 = mybir.dt.float32
    nc = bacc.Bacc(None, target_bir_lowering=False, debug=True)

    xt = nc.dram_tensor("xt", [C, s_bufs], f16, kind="ExternalInput")
    wg = nc.dram_tensor("wg", [C, H], f16, kind="ExternalInput")
    wu = nc.dram_tensor("wu", [C, H], f16, kind="ExternalInput")
    wd = nc.dram_tensor("wd", [H, C], f16, kind="ExternalInput")
    yt = nc.dram_tensor("yt", [C, s_bufs], f32, kind="ExternalOutput")

    nht = len(H_TILES)
    with tile.TileContext(nc) as tc:
        with (
            tc.tile_pool(name="wpool", bufs=1) as wpool,
            tc.tile_pool(name="xpool", bufs=2) as xpool,
            tc.tile_pool(name="apool", bufs=1) as apool,
            tc.tile_pool(name="spool", bufs=2) as spool,
            tc.tile_pool(name="ypool", bufs=4) as ypool,
            tc.tile_pool(name="gup", bufs=2, space="PSUM") as gup,
            tc.tile_pool(name="yp", bufs=1, space="PSUM") as yp,
        ):
            def load_xt_chunk(s0, sw):
                # issued from the scalar queue so startup DMAs don't
                # serialize behind the weight loads on sync (~600 ns/issue)
                tiles = []
                for ct in range(NCT):
                    t = xpool.tile([P, sw], f16, tag=f"x{ct}", name=f"x{ct}")
                    nc.scalar.dma_start(
                        out=t[:], in_=xt[ct * P : (ct + 1) * P, s0 : s0 + sw]
                    )
                    tiles.append(t)
                return tiles

            # PE warm-up: dummy matmuls on a zeroed scratch tile, issued
            # while the first weight/activation DMAs are still in flight so
            # the clock governor un-throttles before real work arrives
            scr = spool.tile([P, 640], f16, tag="warm")
            nc.gpsimd.memset(scr[:], 0.0)
            warm_ps = yp.tile([P, SCH], f32, tag="py0", name="warm")
            for _ in range(24):
                nc.tensor.matmul(
                    warm_ps[:],
                    scr[:, 0:P],
                    scr[:, P : P + SCH],
                    start=True,
                    stop=True,
                    skip_group_check=True,
                )

            # chunk-0 activations first: the first matmul needs them
            xt0 = load_xt_chunk(*s_chunks[0])
            # Wg/Wu in h-ascending slabs (first slab = just h-tile 0) so
            # compute starts after ~1.5 MB of DMA instead of the full 11 MB
            wg_sb = [wpool.tile([P, H_FULL], f16, tag=f"wg{ct}", name=f"wg{ct}") for ct in range(NCT)]
            wu_sb = [wpool.tile([P, H_FULL], f16, tag=f"wu{ct}", name=f"wu{ct}") for ct in range(NCT)]
            slabs, h0 = [], 0
            for hw in (128, 512, 512, 512, 512, 512):
                slabs.append((h0, hw))
                h0 += hw
            assert h0 == H_FULL
            for h0, hw in slabs:
                for ct in range(NCT):
                    nc.sync.dma_start(
                        out=wg_sb[ct][:, h0 : h0 + hw],
                        in_=wg[ct * P : (ct + 1) * P, h0 : h0 + hw],
                    )
                    nc.sync.dma_start(
                        out=wu_sb[ct][:, h0 : h0 + hw],
                        in_=wu[ct * P : (ct + 1) * P, h0 : h0 + hw],
                    )
            # stacked h-remainder: cols 0:64 = Wg[:, H_FULL:], 64:128 = Wu[:, H_FULL:]
            wgu_rem = []
            for ct in range(NCT):
                t = wpool.tile([P, 2 * HREM], f16, tag=f"wr{ct}", name=f"wr{ct}")
                nc.sync.dma_start(
                    out=t[:, 0:HREM], in_=wg[ct * P : (ct + 1) * P, H_FULL:H]
                )
                nc.sync.dma_start(
                    out=t[:, HREM : 2 * HREM],
                    in_=wu[ct * P : (ct + 1) * P, H_FULL:H],
                )
                wgu_rem.append(t)
            wd_sb = []
            for hi, (h0, hsz) in enumerate(H_TILES):
                t = wpool.tile([hsz, C], f16, tag=f"wd{hi}")
                nc.sync.dma_start(out=t[:], in_=wd[h0 : h0 + hsz, :])
                wd_sb.append(t)

            for si, (s0, sw) in enumerate(s_chunks):
                xt_sb = xt0 if si == 0 else load_xt_chunk(s0, sw)
                # --- G/U + SwiGLU epilogue, one h-tile at a time ---
                a_sb = []
                for hi in range(NHF):
                    h0 = hi * P
                    pg = gup.tile([P, SCH], f32, tag="pg")
                    pu = gup.tile([P, SCH], f32, tag="pu")
                    for ct in range(NCT):
                        nc.tensor.matmul(
                            pg[:, :sw],
                            wg_sb[ct][:, h0 : h0 + P],
                            xt_sb[ct][:, :sw],
                            start=(ct == 0),
                            stop=(ct == NCT - 1),
                        )
                    for ct in range(NCT):
                        nc.tensor.matmul(
                            pu[:, :sw],
                            wu_sb[ct][:, h0 : h0 + P],
                            xt_sb[ct][:, :sw],
                            start=(ct == 0),
                            stop=(ct == NCT - 1),
                        )
                    sg = spool.tile([P, sw], f16, tag="sg")
                    nc.scalar.activation(
                        sg[:, :sw],
                        pg[:, :sw],
                        mybir.ActivationFunctionType.Silu,
                    )
                    a = apool.tile([P, sw], f16, tag=f"a{hi}")
                    nc.vector.tensor_mul(a[:, :sw], sg[:, :sw], pu[:, :sw])
                    a_sb.append(a)
                # h remainder: one stacked MM gives G on rows 0:64, U on 64:128
                pgu = gup.tile([P, SCH], f32, tag="pg")
                for ct in range(NCT):
                    nc.tensor.matmul(
                        pgu[:, :sw],
                        wgu_rem[ct][:, :],
                        xt_sb[ct][:, :sw],
                        start=(ct == 0),
                        stop=(ct == NCT - 1),
                    )
                sg = spool.tile([HREM, sw], f16, tag="sgr")
                nc.scalar.activation(
                    sg[:, :sw], pgu[0:HREM, :sw], mybir.ActivationFunctionType.Silu
                )
                a = apool.tile([HREM, sw], f16, tag=f"a{NHF}")
                nc.vector.tensor_mul(
                    a[:, :sw], sg[:, :sw], pgu[HREM : 2 * HREM, :sw]
                )
                a_sb.append(a)
                # --- down-proj: Y^T[c, s] = sum_h Wd[h, c] * A[h, s] ---
                final = si == len(s_chunks) - 1
                for ch in range(2):
                    pys = [
                        yp.tile([P, SCH], f32, tag=f"py{i}", name=f"py{i}")
                        for i in range(4)
                    ]
                    if final and ch == 1:
                        # i-major: each psum group finishes early so its
                        # eviction overlaps the remaining groups' matmuls
                        order = [(hi, i) for i in range(4) for hi in range(nht)]
                    else:
                        order = [(hi, i) for hi in range(nht) for i in range(4)]
                    for hi, i in order:
                        h0, hsz = H_TILES[hi]
                        c0 = ch * 512 + i * P
                        nc.tensor.matmul(
                            pys[i][:, :sw],
                            wd_sb[hi][:hsz, c0 : c0 + P],
                            a_sb[hi][:hsz, :sw],
                            start=(hi == 0),
                            stop=(hi == nht - 1),
                        )
                    for i in range(4):
                        c0 = ch * 512 + i * P
                        ysb = ypool.tile([P, sw], f32, tag="y")
                        nc.vector.tensor_copy(ysb[:, :sw], pys[i][:, :sw])
                        nc.sync.dma_start(
                            out=yt[c0 : c0 + P, s0 : s0 + sw], in_=ysb[:, :sw]
                        )
    nc.compile()
    return nc


def run(inputs, trace=False, tmpdir=None):
    from concourse.bass_utils import run_bass_kernel_spmd

    x = np.ascontiguousarray(np.asarray(inputs["x"], dtype=np.float32))
    gate_w = np.asarray(inputs["gate_w"], dtype=np.float32)
    Wg = np.asarray(inputs["Wg"], dtype=np.float32)
    Wu = np.asarray(inputs["Wu"], dtype=np.float32)
    Wd = np.asarray(inputs["Wd"], dtype=np.float32)

    xf = x.reshape(N, C)
    logits, gates, (i0, i1), (v0, v1) = _route(xf, gate_w)

    # losses (exact host math, fp64 accumulate)
    me = gates.astype(np.float64).mean(axis=0)
    ce = np.bincount(i0, minlength=E).astype(np.float64) / N
    aux_loss = np.float32(E * np.sum(me * ce))
    z_loss = np.float32(np.mean(logits.astype(np.float64) ** 2))

    # dispatch (first-come per k, capacity CAP per (expert, k))
    groups = []  # (tokens, vals) per (k, e)
    counts = np.zeros(E, dtype=np.int64)
    tok_by = {}
    for k, (idx, val) in enumerate(((i0, v0), (i1, v1))):
        pos = _positions(idx)
        keep = pos < CAP
        for e in range(E):
            sel = (idx == e) & keep
            toks = np.nonzero(sel)[0]
            tok_by[(k, e)] = toks
            counts[e] += len(toks)

    s_bufs = max(SCH, int(counts.max()))
    s_chunks = []
    s0 = 0
    while s0 < s_bufs:
        sw = min(SCH, s_bufs - s0)
        s_chunks.append((s0, sw))
        s0 += sw

    xf16 = xf.astype(np.float16)
    in_maps = []
    offsets = {}
    for e in range(E):
        t0 = tok_by[(0, e)]
        t1 = tok_by[(1, e)]
        offsets[e] = (len(t0), len(t1))
        xe = np.zeros((s_bufs, C), np.float16)
        xe[: len(t0)] = xf16[t0]
        xe[len(t0) : len(t0) + len(t1)] = xf16[t1]
        in_maps.append(
            {
                "xt": np.ascontiguousarray(xe.T),
                "wg": Wg[e].astype(np.float16),
                "wu": Wu[e].astype(np.float16),
                "wd": Wd[e].astype(np.float16),
            }
        )

    nc = _build(s_bufs, s_chunks)
    res = run_bass_kernel_spmd(
        nc, in_maps, list(range(E)), trace=trace, tmpdir=tmpdir
    )

    out = np.zeros((N, C), np.float32)
    for e in range(E):
        y = res.results[e]["yt"]  # [C, s_bufs] fp32
        n0, n1 = offsets[e]
        t0 = tok_by[(0, e)]
        t1 = tok_by[(1, e)]
        if n0:
            out[t0] += v0[t0, None] * y[:, :n0].T
        if n1:
            out[t1] += v1[t1, None] * y[:, n0 : n0 + n1].T

    return (out.reshape(B, T, C), aux_loss, z_loss), res


def kernel(**inputs):
    outs, _ = run(inputs)
    return outs


# revision 14
# speedup vs baseline: 1.0230x; 1.0029x over previous
"""MoE (top-2, capacity-dropped, SwiGLU experts) on 8 Trainium2 cores.

Expert-parallel: host computes the (tiny) router + dispatch exactly as the
reference, core e runs the three big GEMMs for expert e in fp16 (fp32
accumulate), host combines with the gate weights.

Self-contained: hardcodes shapes from the problem spec.
"""

import os
import sys

for _p in ("/opt/trn_rl_repo", "/root/.axon_site/_ro/trn_rl_repo"):
    if os.path.isdir(_p) and _p not in sys.path:
        sys.path.append(_p)

import numpy as np

B, T, C, E, H = 8, 2048, 1024, 8, 2752
CAPF = 1.25
N = B * T
CAP = int(CAPF * N * 2 / E)
P = 128
NCT = C // P  # contraction tiles over C
H_TILES = [(i * P, min(P, H - i * P)) for i in range((H + P - 1) // P)]
NHF = H // P  # full 128-row h tiles; the 64-row remainder is handled merged
H_FULL = NHF * P
HREM = H - H_FULL
SCH = 512  # token chunk (matmul free dim / one PSUM bank)


def _route(xf, gate_w):
    """Replicates the reference router in fp32 numpy."""
    logits = xf @ gate_w  # [N, E] fp32
    m = logits.max(axis=-1, keepdims=True)
    eg = np.exp(logits - m)
    gates = eg / eg.sum(axis=-1, keepdims=True)
    i0 = gates.argmax(axis=-1)
    g2 = gates.copy()
    g2[np.arange(N), i0] = -np.inf
    i1 = g2.argmax(axis=-1)
    v0 = gates[np.arange(N), i0]
    v1 = gates[np.arange(N), i1]
    return logits, gates, (i0, i1), (v0, v1)


def _positions(idx):
    """Per-token occurrence index within its chosen expert (token order)."""
    onehot = idx[:, None] == np.arange(E)[None, :]
    return onehot.cumsum(axis=0)[np.arange(N), idx] - 1


def _build(s_bufs, s_chunks):
    import concourse.bacc as bacc
    import concourse.mybir as mybir
    from concourse import tile

    f16 = mybir.dt.float16
    f32 = mybir.dt.float32
    nc = bacc.Bacc(None, target_bir_lowering=False, debug=True)

    # packed layouts (built on host) to minimize DMA descriptor count:
    # xt:  [128, 8*S] — chunk-major, within a chunk ct-major then s
    # wgu: [C, 2*H_FULL] — per h-tile, 128 Wg cols then 128 Wu cols
    # wr:  [C, 128] — Wg h-remainder cols 0:64, Wu h-remainder 64:128
    # wdp: [11*128, 2*C] — h-pair-stacked Wd (second half zero-padded)
    xt = nc.dram_tensor("xt", [P, 8 * s_bufs], f16, kind="ExternalInput")
    wgu = nc.dram_tensor("wgu", [C, 2 * H_FULL], f16, kind="ExternalInput")
    wr = nc.dram_tensor("wr", [C, 2 * HREM], f16, kind="ExternalInput")
    wdp = nc.dram_tensor("wdp", [11 * P, 2 * C], f16, kind="ExternalInput")
    yt = nc.dram_tensor("yt", [C, s_bufs], f32, kind="ExternalOutput")

    nht = len(H_TILES)
    with tile.TileContext(nc) as tc:
        with (
            tc.tile_pool(name="wpool", bufs=1) as wpool,
            tc.tile_pool(name="xpool", bufs=2) as xpool,
            tc.tile_pool(name="apool", bufs=1) as apool,
            tc.tile_pool(name="spool", bufs=2) as spool,
            tc.tile_pool(name="ypool", bufs=4) as ypool,
            tc.tile_pool(name="gup", bufs=2, space="PSUM") as gup,
            tc.tile_pool(name="yp", bufs=1, space="PSUM") as yp,
        ):
            def load_xt_chunk(s0, sw):
                # one packed DMA per chunk, issued from the scalar queue so
                # it doesn't serialize behind the weight loads on sync
                t = xpool.tile([P, 8 * sw], f16, tag="x", name="x")
                nc.scalar.dma_start(out=t[:], in_=xt[:, 8 * s0 : 8 * (s0 + sw)])
                return t

            # PE warm-up: dummy matmuls on a zeroed scratch tile, issued
            # while the first weight/activation DMAs are still in flight so
            # the clock governor un-throttles before real work arrives
            scr = spool.tile([P, 640], f16, tag="warm")
            nc.gpsimd.memset(scr[:], 0.0)
            warm_ps = yp.tile([P, SCH], f32, tag="py0", name="warm")
            for _ in range(24):
                nc.tensor.matmul(
                    warm_ps[:],
                    scr[:, 0:P],
                    scr[:, P : P + SCH],
                    start=True,
                    stop=True,
                    skip_group_check=True,
                )

            # chunk-0 activations first: the first matmul needs them
            xt0 = load_xt_chunk(*s_chunks[0])
            # Wg/Wu interleaved per h-tile, loaded in h-ascending slabs
            # (first slab = just h-tile 0) so compute starts early
            wgu_sb = [
                wpool.tile([P, 2 * H_FULL], f16, tag=f"wgu{ct}", name=f"wgu{ct}")
                for ct in range(NCT)
            ]
            slabs, h0 = [], 0
            for hw in (128, 512, 512, 512, 512, 512):
                slabs.append((h0, hw))
                h0 += hw
            assert h0 == H_FULL
            for h0, hw in slabs:
                for ct in range(NCT):
                    nc.sync.dma_start(
                        out=wgu_sb[ct][:, 2 * h0 : 2 * (h0 + hw)],
                        in_=wgu[ct * P : (ct + 1) * P, 2 * h0 : 2 * (h0 + hw)],
                    )
            # stacked h-remainder: cols 0:64 = Wg[:, H_FULL:], 64:128 = Wu[:, H_FULL:]
            wgu_rem = []
            for ct in range(NCT):
                t = wpool.tile([P, 2 * HREM], f16, tag=f"wr{ct}", name=f"wr{ct}")
                nc.sync.dma_start(out=t[:], in_=wr[ct * P : (ct + 1) * P, :])
                wgu_rem.append(t)
            wd_sb = []
            for q in range(11):
                t = wpool.tile([P, 2 * C], f16, tag=f"wd{q}")
                nc.sync.dma_start(out=t[:], in_=wdp[q * P : (q + 1) * P, :])
                wd_sb.append(t)

            for si, (s0, sw) in enumerate(s_chunks):
                xt_sb = xt0 if si == 0 else load_xt_chunk(s0, sw)
                # --- G/U + SwiGLU epilogue, one h-tile at a time ---
                a_sb = []
                for hi in range(NHF):
                    h0 = hi * P
                    pg = gup.tile([P, SCH], f32, tag="pg")
                    pu = gup.tile([P, SCH], f32, tag="pu")
                    for ct in range(NCT):
                        nc.tensor.matmul(
                            pg[:, :sw],
                            wgu_sb[ct][:, 2 * h0 : 2 * h0 + P],
                            xt_sb[:, ct * sw : (ct + 1) * sw],
                            start=(ct == 0),
                            stop=(ct == NCT - 1),
                        )
                    for ct in range(NCT):
                        nc.tensor.matmul(
                            pu[:, :sw],
                            wgu_sb[ct][:, 2 * h0 + P : 2 * h0 + 2 * P],
                            xt_sb[:, ct * sw : (ct + 1) * sw],
                            start=(ct == 0),
                            stop=(ct == NCT - 1),
                        )
                    sg = spool.tile([P, sw], f16, tag="sg")
                    nc.scalar.activation(
                        sg[:, :sw],
                        pg[:, :sw],
                        mybir.ActivationFunctionType.Silu,
                    )
                    a = apool.tile([P, sw], f16, tag=f"a{hi}")
                    nc.vector.tensor_mul(a[:, :sw], sg[:, :sw], pu[:, :sw])
                    a_sb.append(a)
                # h remainder: one stacked MM gives G on rows 0:64, U on 64:128
                pgu = gup.tile([P, SCH], f32, tag="pg")
                for ct in range(NCT):
                    nc.tensor.matmul(
                        pgu[:, :sw],
                        wgu_rem[ct][:, :],
                        xt_sb[:, ct * sw : (ct + 1) * sw],
                        start=(ct == 0),
                        stop=(ct == NCT - 1),
                    )
                sg = spool.tile([HREM, sw], f16, tag="sgr")
                nc.scalar.activation(
                    sg[:, :sw], pgu[0:HREM, :sw], mybir.ActivationFunctionType.Silu
                )
                a = apool.tile([HREM, sw], f16, tag=f"a{NHF}")
                nc.vector.tensor_mul(
                    a[:, :sw], sg[:, :sw], pgu[HREM : 2 * HREM, :sw]
                )
                a_sb.append(a)
                # --- down-proj: Y^T[c, s] = sum_h Wd[h, c] * A[h, s] ---
                final = si == len(s_chunks) - 1
                for ch in range(2):
                    pys = [
                        yp.tile([P, SCH], f32, tag=f"py{i}", name=f"py{i}")
                        for i in range(4)
                    ]
                    if final and ch == 1:
                        # i-major: each psum group finishes early so its
                        # eviction overlaps the remaining groups' matmuls
                        order = [(hi, i) for i in range(4) for hi in range(nht)]
                    else:
                        order = [(hi, i) for hi in range(nht) for i in range(4)]
                    for hi, i in order:
                        h0, hsz = H_TILES[hi]
                        q, half = divmod(hi, 2)
                        c0 = half * C + ch * 512 + i * P
                        nc.tensor.matmul(
                            pys[i][:, :sw],
                            wd_sb[q][:hsz, c0 : c0 + P],
                            a_sb[hi][:hsz, :sw],
                            start=(hi == 0),
                            stop=(hi == nht - 1),
                        )
                    for i in range(4):
                        c0 = ch * 512 + i * P
                        ysb = ypool.tile([P, sw], f32, tag="y")
                        nc.vector.tensor_copy(ysb[:, :sw], pys[i][:, :sw])
                        nc.sync.dma_start(
                            out=yt[c0 : c0 + P, s0 : s0 + sw], in_=ysb[:, :sw]
                        )
    nc.compile()
    return nc


def run(inputs, trace=False, tmpdir=None):
    from concourse.bass_utils import run_bass_kernel_spmd

    x = np.ascontiguousarray(np.asarray(inputs["x"], dtype=np.float32))
    gate_w = np.asarray(inputs["gate_w"], dtype=np.float32)
    Wg = np.asarray(inputs["Wg"], dtype=np.float32)
    Wu = np.asarray(inputs["Wu"], dtype=np.float32)
    Wd = np.asarray(inputs["Wd"], dtype=np.float32)

    xf = x.reshape(N, C)
    logits, gates, (i0, i1), (v0, v1) = _route(xf, gate_w)

    # losses (exact host math, fp64 accumulate)
    me = gates.astype(np.float64).mean(axis=0)
    ce = np.bincount(i0, minlength=E).astype(np.float64) / N
    aux_loss = np.float32(E * np.sum(me * ce))
    z_loss = np.float32(np.mean(logits.astype(np.float64) ** 2))

    # dispatch (first-come per k, capacity CAP per (expert, k))
    groups = []  # (tokens, vals) per (k, e)
    counts = np.zeros(E, dtype=np.int64)
    tok_by = {}
    for k, (idx, val) in enumerate(((i0, v0), (i1, v1))):
        pos = _positions(idx)
        keep = pos < CAP
        for e in range(E):
            sel = (idx == e) & keep
            toks = np.nonzero(sel)[0]
            tok_by[(k, e)] = toks
            counts[e] += len(toks)

    s_bufs = max(SCH, int(counts.max()))
    s_chunks = []
    s0 = 0
    while s0 < s_bufs:
        sw = min(SCH, s_bufs - s0)
        s_chunks.append((s0, sw))
        s0 += sw

    xf16 = xf.astype(np.float16)
    in_maps = []
    offsets = {}
    for e in range(E):
        t0 = tok_by[(0, e)]
        t1 = tok_by[(1, e)]
        offsets[e] = (len(t0), len(t1))
        xe = np.zeros((s_bufs, C), np.float16)
        xe[: len(t0)] = xf16[t0]
        xe[len(t0) : len(t0) + len(t1)] = xf16[t1]
        # xt packed: per chunk, [128, ct, s] flattened to columns
        xtp = np.empty((P, 8 * s_bufs), np.float16)
        for s0, sw in s_chunks:
            blk = xe[s0 : s0 + sw, :].T.reshape(NCT, P, sw)
            xtp[:, 8 * s0 : 8 * (s0 + sw)] = (
                blk.transpose(1, 0, 2).reshape(P, NCT * sw)
            )
        wg_f = Wg[e].astype(np.float16)
        wu_f = Wu[e].astype(np.float16)
        wd_f = Wd[e].astype(np.float16)
        # wgu: per h-tile 128 Wg cols then 128 Wu cols
        wgu = np.stack(
            [
                wg_f[:, :H_FULL].reshape(C, NHF, P),
                wu_f[:, :H_FULL].reshape(C, NHF, P),
            ],
            axis=2,
        ).reshape(C, 2 * H_FULL)
        wrm = np.concatenate([wg_f[:, H_FULL:], wu_f[:, H_FULL:]], axis=1)
        # wd pair-stacked: pair q rows = h 256q..256q+128 | 256q+128..256q+256
        wdp = np.zeros((11 * P, 2 * C), np.float16)
        for q in range(11):
            wdp[q * P : (q + 1) * P, :C] = wd_f[256 * q : 256 * q + P]
            lo = 256 * q + P
            hsz = min(P, H - lo)
            wdp[q * P : q * P + hsz, C:] = wd_f[lo : lo + hsz]
        in_maps.append(
            {
                "xt": np.ascontiguousarray(xtp),
                "wgu": np.ascontiguousarray(wgu),
                "wr": np.ascontiguousarray(wrm),
                "wdp": wdp,
            }
        )

    nc = _build(s_bufs, s_chunks)
    res = run_bass_kernel_spmd(
        nc, in_maps, list(range(E)), trace=trace, tmpdir=tmpdir
    )

    out = np.zeros((N, C), np.float32)
    for e in range(E):
        y = res.results[e]["yt"]  # [C, s_bufs] fp32
        n0, n1 = offsets[e]
        t0 = tok_by[(0, e)]
        t1 = tok_by[(1, e)]
        if n0:
            out[t0] += v0[t0, None] * y[:, :n0].T
        if n1:
            out[t1] += v1[t1, None] * y[:, n0 : n0 + n1].T

    return (out.reshape(B, T, C), aux_loss, z_loss), res


def kernel(**inputs):
    outs, _ = run(inputs)
    return outs


# revision 15
# speedup vs baseline: 1.0254x; 1.0024x over previous
"""MoE (top-2, capacity-dropped, SwiGLU experts) on 8 Trainium2 cores.

Expert-parallel: host computes the (tiny) router + dispatch exactly as the
reference, core e runs the three big GEMMs for expert e in fp16 (fp32
accumulate), host combines with the gate weights.

Self-contained: hardcodes shapes from the problem spec.
"""

import os
import sys

for _p in ("/opt/trn_rl_repo", "/root/.axon_site/_ro/trn_rl_repo"):
    if os.path.isdir(_p) and _p not in sys.path:
        sys.path.append(_p)

import numpy as np

B, T, C, E, H = 8, 2048, 1024, 8, 2752
CAPF = 1.25
N = B * T
CAP = int(CAPF * N * 2 / E)
P = 128
NCT = C // P  # contraction tiles over C
H_TILES = [(i * P, min(P, H - i * P)) for i in range((H + P - 1) // P)]
NHF = H // P  # full 128-row h tiles; the 64-row remainder is handled merged
H_FULL = NHF * P
HREM = H - H_FULL
SCH = 512  # token chunk (matmul free dim / one PSUM bank)


def _route(xf, gate_w):
    """Replicates the reference router in fp32 numpy."""
    logits = xf @ gate_w  # [N, E] fp32
    m = logits.max(axis=-1, keepdims=True)
    eg = np.exp(logits - m)
    gates = eg / eg.sum(axis=-1, keepdims=True)
    i0 = gates.argmax(axis=-1)
    g2 = gates.copy()
    g2[np.arange(N), i0] = -np.inf
    i1 = g2.argmax(axis=-1)
    v0 = gates[np.arange(N), i0]
    v1 = gates[np.arange(N), i1]
    return logits, gates, (i0, i1), (v0, v1)


def _positions(idx):
    """Per-token occurrence index within its chosen expert (token order)."""
    onehot = idx[:, None] == np.arange(E)[None, :]
    return onehot.cumsum(axis=0)[np.arange(N), idx] - 1


def _build(s_bufs, s_chunks):
    import concourse.bacc as bacc
    import concourse.mybir as mybir
    from concourse import tile

    f16 = mybir.dt.float16
    f32 = mybir.dt.float32
    nc = bacc.Bacc(None, target_bir_lowering=False, debug=True)

    # packed layouts (built on host) to minimize DMA descriptor count:
    # xt:  [128, 8*S] — chunk-major, within a chunk ct-major then s
    # wgu: [C, 2*H_FULL] — per h-tile, 128 Wg cols then 128 Wu cols
    # wr:  [C, 128] — Wg h-remainder cols 0:64, Wu h-remainder 64:128
    # wdp: [11*128, 2*C] — h-pair-stacked Wd (second half zero-padded)
    xt = nc.dram_tensor("xt", [P, 8 * s_bufs], f16, kind="ExternalInput")
    wgu = nc.dram_tensor("wgu", [C, 2 * H_FULL], f16, kind="ExternalInput")
    wr = nc.dram_tensor("wr", [C, 2 * HREM], f16, kind="ExternalInput")
    wdp = nc.dram_tensor("wdp", [11 * P, 2 * C], f16, kind="ExternalInput")
    yt = nc.dram_tensor("yt", [C, s_bufs], f32, kind="ExternalOutput")

    nht = len(H_TILES)
    with tile.TileContext(nc) as tc:
        with (
            tc.tile_pool(name="wpool", bufs=1) as wpool,
            tc.tile_pool(name="xpool", bufs=2) as xpool,
            tc.tile_pool(name="apool", bufs=1) as apool,
            tc.tile_pool(name="spool", bufs=2) as spool,
            tc.tile_pool(name="ypool", bufs=4) as ypool,
            tc.tile_pool(name="gup", bufs=2, space="PSUM") as gup,
            tc.tile_pool(name="yp", bufs=1, space="PSUM") as yp,
        ):
            def load_xt_chunk(s0, sw):
                # one packed DMA per chunk, issued from the scalar queue so
                # it doesn't serialize behind the weight loads on sync
                t = xpool.tile([P, 8 * sw], f16, tag="x", name="x")
                nc.scalar.dma_start(out=t[:], in_=xt[:, 8 * s0 : 8 * (s0 + sw)])
                return t

            # PE warm-up: dummy matmuls on a zeroed scratch tile, issued
            # while the first weight/activation DMAs are still in flight so
            # the clock governor un-throttles before real work arrives
            scr = spool.tile([P, 640], f16, tag="warm")
            nc.gpsimd.memset(scr[:], 0.0)
            warm_ps = yp.tile([P, SCH], f32, tag="py0", name="warm")
            for _ in range(32):
                nc.tensor.matmul(
                    warm_ps[:],
                    scr[:, 0:P],
                    scr[:, P : P + SCH],
                    start=True,
                    stop=True,
                    skip_group_check=True,
                )

            # chunk-0 activations first: the first matmul needs them
            xt0 = load_xt_chunk(*s_chunks[0])
            # Wg/Wu interleaved per h-tile, loaded in h-ascending slabs
            # (first slab = just h-tile 0) so compute starts early
            wgu_sb = [
                wpool.tile([P, 2 * H_FULL], f16, tag=f"wgu{ct}", name=f"wgu{ct}")
                for ct in range(NCT)
            ]
            slabs, h0 = [], 0
            for hw in (128, 512, 512, 512, 512, 512):
                slabs.append((h0, hw))
                h0 += hw
            assert h0 == H_FULL
            for h0, hw in slabs:
                for ct in range(NCT):
                    nc.sync.dma_start(
                        out=wgu_sb[ct][:, 2 * h0 : 2 * (h0 + hw)],
                        in_=wgu[ct * P : (ct + 1) * P, 2 * h0 : 2 * (h0 + hw)],
                    )
            # stacked h-remainder: cols 0:64 = Wg[:, H_FULL:], 64:128 = Wu[:, H_FULL:]
            wgu_rem = []
            for ct in range(NCT):
                t = wpool.tile([P, 2 * HREM], f16, tag=f"wr{ct}", name=f"wr{ct}")
                nc.sync.dma_start(out=t[:], in_=wr[ct * P : (ct + 1) * P, :])
                wgu_rem.append(t)
            wd_sb = []
            for q in range(11):
                t = wpool.tile([P, 2 * C], f16, tag=f"wd{q}")
                nc.sync.dma_start(out=t[:], in_=wdp[q * P : (q + 1) * P, :])
                wd_sb.append(t)

            for si, (s0, sw) in enumerate(s_chunks):
                xt_sb = xt0 if si == 0 else load_xt_chunk(s0, sw)
                # --- G/U + SwiGLU epilogue, one h-tile at a time ---
                a_sb = []
                for hi in range(NHF):
                    h0 = hi * P
                    pg = gup.tile([P, SCH], f32, tag="pg")
                    pu = gup.tile([P, SCH], f32, tag="pu")
                    for ct in range(NCT):
                        nc.tensor.matmul(
                            pg[:, :sw],
                            wgu_sb[ct][:, 2 * h0 : 2 * h0 + P],
                            xt_sb[:, ct * sw : (ct + 1) * sw],
                            start=(ct == 0),
                            stop=(ct == NCT - 1),
                        )
                    for ct in range(NCT):
                        nc.tensor.matmul(
                            pu[:, :sw],
                            wgu_sb[ct][:, 2 * h0 + P : 2 * h0 + 2 * P],
                            xt_sb[:, ct * sw : (ct + 1) * sw],
                            start=(ct == 0),
                            stop=(ct == NCT - 1),
                        )
                    sg = spool.tile([P, sw], f16, tag="sg")
                    nc.scalar.activation(
                        sg[:, :sw],
                        pg[:, :sw],
                        mybir.ActivationFunctionType.Silu,
                    )
                    a = apool.tile([P, sw], f16, tag=f"a{hi}")
                    nc.vector.tensor_mul(a[:, :sw], sg[:, :sw], pu[:, :sw])
                    a_sb.append(a)
                # h remainder: one stacked MM gives G on rows 0:64, U on 64:128
                pgu = gup.tile([P, SCH], f32, tag="pg")
                for ct in range(NCT):
                    nc.tensor.matmul(
                        pgu[:, :sw],
                        wgu_rem[ct][:, :],
                        xt_sb[:, ct * sw : (ct + 1) * sw],
                        start=(ct == 0),
                        stop=(ct == NCT - 1),
                    )
                sg = spool.tile([HREM, sw], f16, tag="sgr")
                nc.scalar.activation(
                    sg[:, :sw], pgu[0:HREM, :sw], mybir.ActivationFunctionType.Silu
                )
                a = apool.tile([HREM, sw], f16, tag=f"a{NHF}")
                nc.vector.tensor_mul(
                    a[:, :sw], sg[:, :sw], pgu[HREM : 2 * HREM, :sw]
                )
                a_sb.append(a)
                # --- down-proj: Y^T[c, s] = sum_h Wd[h, c] * A[h, s] ---
                final = si == len(s_chunks) - 1
                for ch in range(2):
                    pys = [
                        yp.tile([P, SCH], f32, tag=f"py{i}", name=f"py{i}")
                        for i in range(4)
                    ]
                    if final and ch == 1:
                        # i-major: each psum group finishes early so its
                        # eviction overlaps the remaining groups' matmuls
                        order = [(hi, i) for i in range(4) for hi in range(nht)]
                    else:
                        order = [(hi, i) for hi in range(nht) for i in range(4)]
                    for hi, i in order:
                        h0, hsz = H_TILES[hi]
                        q, half = divmod(hi, 2)
                        c0 = half * C + ch * 512 + i * P
                        nc.tensor.matmul(
                            pys[i][:, :sw],
                            wd_sb[q][:hsz, c0 : c0 + P],
                            a_sb[hi][:hsz, :sw],
                            start=(hi == 0),
                            stop=(hi == nht - 1),
                        )
                    for i in range(4):
                        c0 = ch * 512 + i * P
                        ysb = ypool.tile([P, sw], f32, tag="y")
                        nc.vector.tensor_copy(ysb[:, :sw], pys[i][:, :sw])
                        nc.sync.dma_start(
                            out=yt[c0 : c0 + P, s0 : s0 + sw], in_=ysb[:, :sw]
                        )
    nc.compile()
    return nc


def run(inputs, trace=False, tmpdir=None):
    from concourse.bass_utils import run_bass_kernel_spmd

    x = np.ascontiguousarray(np.asarray(inputs["x"], dtype=np.float32))
    gate_w = np.asarray(inputs["gate_w"], dtype=np.float32)
    Wg = np.asarray(inputs["Wg"], dtype=np.float32)
    Wu = np.asarray(inputs["Wu"], dtype=np.float32)
    Wd = np.asarray(inputs["Wd"], dtype=np.float32)

    xf = x.reshape(N, C)
    logits, gates, (i0, i1), (v0, v1) = _route(xf, gate_w)

    # losses (exact host math, fp64 accumulate)
    me = gates.astype(np.float64).mean(axis=0)
    ce = np.bincount(i0, minlength=E).astype(np.float64) / N
    aux_loss = np.float32(E * np.sum(me * ce))
    z_loss = np.float32(np.mean(logits.astype(np.float64) ** 2))

    # dispatch (first-come per k, capacity CAP per (expert, k))
    groups = []  # (tokens, vals) per (k, e)
    counts = np.zeros(E, dtype=np.int64)
    tok_by = {}
    for k, (idx, val) in enumerate(((i0, v0), (i1, v1))):
        pos = _positions(idx)
        keep = pos < CAP
        for e in range(E):
            sel = (idx == e) & keep
            toks = np.nonzero(sel)[0]
            tok_by[(k, e)] = toks
            counts[e] += len(toks)

    s_bufs = max(SCH, int(counts.max()))
    s_chunks = []
    s0 = 0
    while s0 < s_bufs:
        sw = min(SCH, s_bufs - s0)
        s_chunks.append((s0, sw))
        s0 += sw

    xf16 = xf.astype(np.float16)
    in_maps = []
    offsets = {}
    for e in range(E):
        t0 = tok_by[(0, e)]
        t1 = tok_by[(1, e)]
        offsets[e] = (len(t0), len(t1))
        xe = np.zeros((s_bufs, C), np.float16)
        xe[: len(t0)] = xf16[t0]
        xe[len(t0) : len(t0) + len(t1)] = xf16[t1]
        # xt packed: per chunk, [128, ct, s] flattened to columns
        xtp = np.empty((P, 8 * s_bufs), np.float16)
        for s0, sw in s_chunks:
            blk = xe[s0 : s0 + sw, :].T.reshape(NCT, P, sw)
            xtp[:, 8 * s0 : 8 * (s0 + sw)] = (
                blk.transpose(1, 0, 2).reshape(P, NCT * sw)
            )
        wg_f = Wg[e].astype(np.float16)
        wu_f = Wu[e].astype(np.float16)
        wd_f = Wd[e].astype(np.float16)
        # wgu: per h-tile 128 Wg cols then 128 Wu cols
        wgu = np.stack(
            [
                wg_f[:, :H_FULL].reshape(C, NHF, P),
                wu_f[:, :H_FULL].reshape(C, NHF, P),
            ],
            axis=2,
        ).reshape(C, 2 * H_FULL)
        wrm = np.concatenate([wg_f[:, H_FULL:], wu_f[:, H_FULL:]], axis=1)
        # wd pair-stacked: pair q rows = h 256q..256q+128 | 256q+128..256q+256
        wdp = np.zeros((11 * P, 2 * C), np.float16)
        for q in range(11):
            wdp[q * P : (q + 1) * P, :C] = wd_f[256 * q : 256 * q + P]
            lo = 256 * q + P
            hsz = min(P, H - lo)
            wdp[q * P : q * P + hsz, C:] = wd_f[lo : lo + hsz]
        in_maps.append(
            {
                "xt": np.ascontiguousarray(xtp),
                "wgu": np.ascontiguousarray(wgu),
                "wr": np.ascontiguousarray(wrm),
                "wdp": wdp,
            }
        )

    nc = _build(s_bufs, s_chunks)
    res = run_bass_kernel_spmd(
        nc, in_maps, list(range(E)), trace=trace, tmpdir=tmpdir
    )

    out = np.zeros((N, C), np.float32)
    for e in range(E):
        y = res.results[e]["yt"]  # [C, s_bufs] fp32
        n0, n1 = offsets[e]
        t0 = tok_by[(0, e)]
        t1 = tok_by[(1, e)]
        if n0:
            out[t0] += v0[t0, None] * y[:, :n0].T
        if n1:
            out[t1] += v1[t1, None] * y[:, n0 : n0 + n1].T

    return (out.reshape(B, T, C), aux_loss, z_loss), res


def kernel(**inputs):
    outs, _ = run(inputs)
    return outs
